# revision 1
# baseline (speedup 1.0000x reference)
"""Trainium2 Bass kernel: cached causal self-attention (dense transformer block).

Full module: y = CausalAttn(x; Wq, Wk, Wv) @ Wo.T + bo with
  B=4, S=2048, E=2048, H=16 heads, Dh=128, fp32 inputs.

Distribution: 8-way tensor parallel over heads (2 heads per NeuronCore).
Each core computes Q/K/V projections for its 2 heads (contraction over the
full embedding dim), causal-softmax attention for those heads, and a partial
output projection y_c = ctx_c @ Wo[:, c*256:(c+1)*256].T.  The host sums the
8 partials and adds the bias (the cross-head reduction of the output
projection), avoiding on-device collectives.

Matmuls run in float32r (single-pass fp32 on the PE array, ~1e-4 relative
error, 4x the throughput of exact fp32).  Layout choices:
  - x is pre-transposed on the host (xT [E, B*S]) so every contraction over
    E has E on the SBUF partition dim with clean contiguous DMAs.
  - scores are computed transposed (sT[k, q]) so no on-chip transpose of the
    attention matrix is ever needed: exp(sT) feeds the attn@V matmul as the
    moving operand directly (ctxT[d, q] = sum_k V[k, d]^T-free exp(sT)[k, q]).
  - softmax denominators (column sums of exp(sT)) come from a ones-vector
    matmul; they are re-laid-out to [s partitions, 1] via a tiny DMA fold +
    PE transpose so the normalization happens per-partition at the output
    projection eviction (per head, before the two heads' partials mix).
"""

import math

import ml_dtypes
import numpy as np

import concourse.bacc as bacc
import concourse.mybir as mybir
import concourse.tile as tile
from concourse.bass_utils import run_bass_kernel_spmd

F32 = mybir.dt.float32
F32R = mybir.dt.float32r
AF = mybir.ActivationFunctionType
ALU = mybir.AluOpType

NEG = -1.0e30

# Full-problem constants
EMB = 2048
N_HEADS = 16
HEAD_DIM = 128
B_FULL = 4
S_FULL = 2048
N_CORES = 8
HPC = N_HEADS // N_CORES  # heads per core = 2


def build(B=B_FULL, S=S_FULL, E=EMB, hpc=HPC, DH=HEAD_DIM, CH=512, reps=1):
    """Build the per-core Bass program (same program on all 8 cores)."""
    assert hpc == 2, "y eviction chain is written for 2 heads per core"
    SB = B * S
    DHC = hpc * DH          # per-core head dims (256)
    NE = E // 128           # e-tiles (contraction tiles)
    NCH = S // CH           # 512-wide chunks per sequence
    KPC = CH // 128         # k-tiles per chunk (4)
    NST = S // 128          # 128-row s-tiles per sequence
    NOC = E // CH           # output chunks
    scale = 1.0 / math.sqrt(DH)

    nc = bacc.Bacc("TRN2", target_bir_lowering=False, debug=False,
                   num_devices=N_CORES)

    xT = nc.dram_tensor("xT", [E, SB], F32R, kind="ExternalInput")
    wqT = nc.dram_tensor("wqT", [E, DHC], F32R, kind="ExternalInput")
    wkT = nc.dram_tensor("wkT", [E, DHC], F32R, kind="ExternalInput")
    wvT = nc.dram_tensor("wvT", [E, DHC], F32R, kind="ExternalInput")
    woT = nc.dram_tensor("woT", [DHC, E], F32R, kind="ExternalInput")
    masks = nc.dram_tensor("masks", [128, CH], mybir.dt.bfloat16, kind="ExternalInput")
    ones = nc.dram_tensor("ones", [128, 1], F32R, kind="ExternalInput")
    y = nc.dram_tensor("y", [SB, E], F32, kind="ExternalOutput")

    with tile.TileContext(nc) as tc:
        with (
            tc.tile_pool(name="wpool", bufs=1) as wpool,
            tc.tile_pool(name="xtp", bufs=2) as xtp,
            tc.tile_pool(name="qkv", bufs=1) as qkv,
            tc.tile_pool(name="expp", bufs=4) as expp,
            tc.tile_pool(name="denp", bufs=1) as denp_sb,
            tc.tile_pool(name="dramp", bufs=2, space="DRAM") as dramp,
            tc.tile_pool(name="yout", bufs=2) as yout,
            tc.tile_pool(name="ps_mm", bufs=3, space="PSUM") as ps_mm,
            tc.tile_pool(name="ps_proj", bufs=2, space="PSUM") as ps_proj,
            tc.tile_pool(name="ps_av", bufs=2, space="PSUM") as ps_av,
            tc.tile_pool(name="ps_den", bufs=1, space="PSUM") as ps_den,
        ):
            # Resident weights / constants (one batched DMA each)
            wq_sb = wpool.tile([128, NE, DHC], F32R, tag="wq")
            wk_sb = wpool.tile([128, NE, DHC], F32R, tag="wk")
            wv_sb = wpool.tile([128, NE, DHC], F32R, tag="wv")
            wo_sb = wpool.tile([128, hpc, E], F32R, tag="wo")
            xT_r = xT.rearrange("(t p) s -> p t s", p=128)
            NEH = NE // 2
            wq_r = wqT.rearrange("(t p) d -> p t d", p=128)
            wk_r = wkT.rearrange("(t p) d -> p t d", p=128)
            # halves: the first Q/K accumulation consumes e-tiles in order,
            # so the low half arriving first starts the PE sooner
            nc.sync.dma_start(wq_sb[:, 0:NE // 2, :], wq_r[:, 0:NE // 2, :])
            nc.sync.dma_start(wk_sb[:, 0:NE // 2, :], wk_r[:, 0:NE // 2, :])
            nc.sync.dma_start(wq_sb[:, NE // 2:NE, :], wq_r[:, NE // 2:NE, :])
            nc.sync.dma_start(wk_sb[:, NE // 2:NE, :], wk_r[:, NE // 2:NE, :])
            xpre0 = None
            if reps == 1:
                # prefetch the first x chunk ahead of the remaining (not yet
                # needed) weights so the first Q matmuls start ~20us sooner
                x0a = xtp.tile([128, NEH, CH], F32R, tag="xta", name="x0a")
                nc.sync.dma_start(x0a[:], xT_r[:, 0:NEH, 0:CH])
                x0b = xtp.tile([128, NEH, CH], F32R, tag="xtb", name="x0b")
                nc.sync.dma_start(x0b[:], xT_r[:, NEH:NE, 0:CH])
                xpre0 = ((0, 0), x0a, x0b)
            nc.sync.dma_start(wv_sb[:], wvT.rearrange("(t p) d -> p t d", p=128))
            nc.sync.dma_start(wo_sb[:], woT.rearrange("(h p) e -> p h e", p=128))
            mask_sb = wpool.tile([128, CH], mybir.dt.bfloat16, tag="mask")
            nc.sync.dma_start(mask_sb[:], masks[:, :])
            ones_sb = wpool.tile([128, 1], F32R, tag="ones")
            nc.sync.dma_start(ones_sb[:], ones[:, :])

            import contextlib
            rep_cm = tc.For_i(0, reps, 1) if reps > 1 else contextlib.nullcontext()
            with rep_cm:
              pending_proj = None
              for b in range(B):
                  s0 = b * S
                  # ---------------- Phase A: Q/K/V projections -------------
                  qT = qkv.tile([128, hpc, S], F32R, tag="qT")
                  kT = qkv.tile([128, hpc, S], F32R, tag="kT")
                  v_sb = qkv.tile([128, NST, DHC], F32R, tag="v")
                  if b == 0:
                      xpre = xpre0
                  for ch in range(NCH):
                      c0 = ch * CH
                      if xpre is not None and xpre[0] == (b, ch):
                          xta, xtb = xpre[1], xpre[2]
                      else:
                          xta = xtp.tile([128, NEH, CH], F32R, tag="xta")
                          nc.sync.dma_start(xta[:],
                                            xT_r[:, 0:NEH, s0 + c0:s0 + c0 + CH])
                          xtb = xtp.tile([128, NEH, CH], F32R, tag="xtb")
                          nc.sync.dma_start(xtb[:],
                                            xT_r[:, NEH:NE, s0 + c0:s0 + c0 + CH])
                      if ch + 1 < NCH or b + 1 < B:
                          nb_, nch = (b, ch + 1) if ch + 1 < NCH else (b + 1, 0)
                          n0 = nb_ * S + nch * CH
                          xna = xtp.tile([128, NEH, CH], F32R, tag="xta",
                                         name="xna")
                          nc.sync.dma_start(xna[:], xT_r[:, 0:NEH, n0:n0 + CH])
                          xnb = xtp.tile([128, NEH, CH], F32R, tag="xtb",
                                         name="xnb")
                          nc.sync.dma_start(xnb[:], xT_r[:, NEH:NE, n0:n0 + CH])
                          xpre = ((nb_, nch), xna, xnb)
                      else:
                          xpre = None

                      def xslice(et, lo=None, hi=None):
                          t = xta if et < NEH else xtb
                          e = et if et < NEH else et - NEH
                          if lo is None:
                              return t[:, e, :]
                          return t[:, e, lo:hi]

                      for h in range(hpc):
                          qp = ps_mm.tile([128, CH], F32, tag="qkvp")
                          for et in range(NE):
                              nc.tensor.matmul(
                                  qp[:], wq_sb[:, et, h * DH:(h + 1) * DH],
                                  xslice(et),
                                  start=(et == 0), stop=(et == NE - 1))
                          nc.scalar.activation(qT[:, h, c0:c0 + CH], qp[:],
                                               AF.Identity, scale=scale)
                          kp = ps_mm.tile([128, CH], F32, tag="qkvp")
                          for et in range(NE):
                              nc.tensor.matmul(
                                  kp[:], wk_sb[:, et, h * DH:(h + 1) * DH],
                                  xslice(et),
                                  start=(et == 0), stop=(et == NE - 1))
                          nc.scalar.activation(kT[:, h, c0:c0 + CH], kp[:], AF.Identity)
                      for st in range(KPC):
                          vp = ps_mm.tile([128, DHC], F32, tag="qkvp")
                          for et in range(NE):
                              nc.tensor.matmul(
                                  vp[:], xslice(et, st * 128, (st + 1) * 128),
                                  wv_sb[:, et, :],
                                  start=(et == 0), stop=(et == NE - 1))
                          nc.scalar.activation(v_sb[:, ch * KPC + st, :], vp[:],
                                               AF.Identity)

                  if pending_proj is not None:
                      emit_proj(*pending_proj)
                      pending_proj = None

                  # ------- Phase B+C: attention with interleaved projection ----
                  # Per 512-chunk g: both heads' attention for queries in g,
                  # per-chunk softmax denominators, then the output projection
                  # for chunk g's s-tiles.  This spreads the y write-out DMA
                  # into the attention window (which has no DMA traffic of its
                  # own) instead of bunching it at the end of the batch.
                  ctxT = qkv.tile([128, hpc, S], F32R, tag="ctxT")
                  rdenT = [denp_sb.tile([128, NST], F32, tag=f"rden{h}",
                                        name=f"rden{h}")
                           for h in range(hpc)]
                  def emit_proj(pctx, prden, ps0, g):
                      for st in range(g * KPC, (g + 1) * KPC):
                          for oc in range(NOC):
                              o0 = oc * CH
                              p0 = ps_proj.tile([128, CH], F32, tag="proj")
                              nc.tensor.matmul(
                                  p0[:], pctx[:, 0, st * 128:(st + 1) * 128],
                                  wo_sb[:, 0, o0:o0 + CH], start=True, stop=True)
                              p1 = ps_proj.tile([128, CH], F32, tag="proj")
                              nc.tensor.matmul(
                                  p1[:], pctx[:, 1, st * 128:(st + 1) * 128],
                                  wo_sb[:, 1, o0:o0 + CH], start=True, stop=True)
                              ysb = yout.tile([128, CH], F32, tag="ysb")
                              if (st + oc) % 2 == 0:
                                  nc.scalar.activation(
                                      ysb[:], p0[:], AF.Identity,
                                      scale=prden[0][:, st:st + 1])
                              else:
                                  nc.vector.tensor_scalar(
                                      ysb[:], p0[:], prden[0][:, st:st + 1],
                                      None, op0=ALU.mult)
                              nc.vector.scalar_tensor_tensor(
                                  ysb[:], p1[:], prden[1][:, st:st + 1], ysb[:],
                                  op0=ALU.mult, op1=ALU.add)
                              nc.gpsimd.dma_start(
                                  y[ps0 + st * 128:ps0 + (st + 1) * 128,
                                    o0:o0 + CH], ysb[:])

                  for g in range(NCH):
                      for h in range(hpc):
                          nk = KPC * (g + 1)
                          avp = ps_av.tile([128, CH], F32, tag="av")
                          dnp = ps_den.tile([1, CH], F32, tag="den")
                          for kt in range(nk):
                              # diagonal k-tiles: queries q < 128j are fully
                              # masked -- compute only the suffix [off, CH)
                              j = kt - (nk - KPC)
                              off = 128 * j if j > 0 else 0
                              w = CH - off
                              sp = ps_mm.tile([128, CH], F32, tag="qkvp", name="sp")
                              nc.tensor.matmul(
                                  sp[:, off:], kT[:, h, kt * 128:(kt + 1) * 128],
                                  qT[:, h, g * CH + off:(g + 1) * CH],
                                  start=True, stop=True)
                              if j >= 0:
                                  # mask col c: masked iff c < p (strict tri)
                                  nc.vector.tensor_add(sp[:, off:], sp[:, off:],
                                                       mask_sb[:, 0:w])
                              ex = expp.tile([128, CH], F32R, tag="ex")
                              nc.scalar.activation(ex[:, off:], sp[:, off:], AF.Exp)
                              nc.tensor.matmul(
                                  avp[:, off:], v_sb[:, kt, h * DH:(h + 1) * DH],
                                  ex[:, off:],
                                  start=(kt == 0), stop=(kt == nk - 1),
                                  skip_group_check=True)
                              nc.tensor.matmul(
                                  dnp[:, off:], ones_sb[:], ex[:, off:],
                                  start=(kt == 0), stop=(kt == nk - 1),
                                  skip_group_check=True)
                          nc.scalar.activation(ctxT[:, h, g * CH:(g + 1) * CH],
                                               avp[:], AF.Identity)
                          den_ch = denp_sb.tile([1, CH], F32, tag="den_ch")
                          nc.scalar.activation(den_ch[:], dnp[:], AF.Identity)
                          # bounce the 2KB denominator row through DRAM to
                          # transpose it to [128 s-partitions, KPC] with pure
                          # address-stream APs, all on the idle Pool engine --
                          # the PE never sits in the denominator chain
                          den_d = dramp.tile([1, CH], F32, tag="den_d")
                          nc.gpsimd.dma_start(den_d[:], den_ch[:])
                          den_t = denp_sb.tile([128, KPC], F32, tag="den_t")
                          nc.gpsimd.dma_start(
                              den_t[:],
                              den_d[:].rearrange("p (j q) -> (p q) j", j=KPC))
                          nc.vector.reciprocal(
                              rdenT[h][:, g * KPC:(g + 1) * KPC], den_t[:])
                      if g > 0:
                          emit_proj(ctxT, rdenT, s0, g - 1)
                  pending_proj = (ctxT, rdenT, s0, NCH - 1)
              if pending_proj is not None:
                  emit_proj(*pending_proj)
                  pending_proj = None
    nc.finalize()
    return nc


def host_consts(S=S_FULL, CH=512):
    """Mask / ones / identity constant inputs."""
    KPC = CH // 128
    NST = S // 128
    p = np.arange(128)[:, None]
    c = np.arange(CH)[None, :]
    # strict lower triangle: masked iff c < p (diagonal k-tile suffix mask)
    masks = np.where(c < p, np.float32(NEG), np.float32(0.0))
    masks = np.ascontiguousarray(masks.astype(ml_dtypes.bfloat16))
    return {
        "masks": masks,
        "ones": np.ones((128, 1), dtype=np.float32),
    }


def host_inputs(x, Wq, Wk, Wv, Wo, B=B_FULL, S=S_FULL, E=EMB, hpc=HPC,
                DH=HEAD_DIM, CH=512):
    """Shard + lay out the full inputs for the 8 cores."""
    SB = B * S
    DHC = hpc * DH
    xT = np.ascontiguousarray(x.reshape(SB, E).T)
    consts = host_consts(S, CH)

    in_maps = []
    for c in range(N_CORES):
        lo, hi = c * DHC, (c + 1) * DHC
        in_maps.append({
            "xT": xT,
            "wqT": np.ascontiguousarray(Wq[lo:hi, :].T),
            "wkT": np.ascontiguousarray(Wk[lo:hi, :].T),
            "wvT": np.ascontiguousarray(Wv[lo:hi, :].T),
            "woT": np.ascontiguousarray(Wo[:, lo:hi].T),
            **consts,
        })
    return in_maps


def kernel(x, Wq, Wk, Wv, Wo, bo):
    x = np.asarray(x, dtype=np.float32)
    Wq = np.asarray(Wq, dtype=np.float32)
    Wk = np.asarray(Wk, dtype=np.float32)
    Wv = np.asarray(Wv, dtype=np.float32)
    Wo = np.asarray(Wo, dtype=np.float32)
    bo = np.asarray(bo, dtype=np.float32)

    nc = build()
    in_maps = host_inputs(x, Wq, Wk, Wv, Wo)
    res = run_bass_kernel_spmd(nc, in_maps, list(range(N_CORES)))
    y = res.results[0]["y"].astype(np.float64)
    for c in range(1, N_CORES):
        y += res.results[c]["y"]
    y = (y + bo).astype(np.float32)
    return y.reshape(B_FULL, S_FULL, EMB)



# revision 3
# speedup vs baseline: 1.0572x; 1.0572x over previous
"""Trainium2 Bass kernel: cached causal self-attention (dense transformer block).

Full module: y = CausalAttn(x; Wq, Wk, Wv) @ Wo.T + bo with
  B=4, S=2048, E=2048, H=16 heads, Dh=128, fp32 inputs.

Distribution: 8-way tensor parallel over heads (2 heads per NeuronCore).
Each core computes Q/K/V projections for its 2 heads, causal-softmax
attention, and a partial output projection; the host sums the 8 partials
and adds the bias.

All matmul operands are bf16 (PSUM accumulation stays fp32): same PE
streaming rate as float32r but Fast-Weight-Load halves the LDWEIGHTS
cost, SBUF/DMA traffic halves, and power throttling drops.  End-to-end
rel err ~5e-3 (validated offline), well inside the 2e-2 gate.

Layout: x pre-transposed on host (xT [E, B*S]); scores computed
transposed (sT[k, q]) so exp(sT) feeds attn@V directly with no on-chip
transpose.  Softmax denominators come from a ones-vector matmul
accumulated in PSUM.

Schedule (the point of this version): the two heads' attention k-tile
loops are interleaved so the PE never waits on the scalar engine's exp
 -- per k-tile step the PE runs s0,s1,[proj filler],av0,dn0,av1,dn1
while ACT runs exp0,exp1 of the previous step's scores.  The output
projection accumulates BOTH heads into one PSUM bank (ctx is
pre-normalized by 1/den), turning the eviction into a plain copy, and
its matmul pairs are spread through the attention steps and the next
batch's QKV phase as PE filler work.  1/den is produced per (g,h) by a
DMA round-trip through DRAM (transpose to [128,4], DVE reciprocal,
transpose back, partition-broadcast) entirely on the idle gpsimd/DMA
path.
"""

import math

import ml_dtypes
import numpy as np

import concourse.bacc as bacc
import concourse.mybir as mybir
import concourse.tile as tile
from concourse.bass_utils import run_bass_kernel_spmd

F32 = mybir.dt.float32
BF16 = mybir.dt.bfloat16
AF = mybir.ActivationFunctionType
ALU = mybir.AluOpType

NEG = -1.0e30

# Full-problem constants
EMB = 2048
N_HEADS = 16
HEAD_DIM = 128
B_FULL = 4
S_FULL = 2048
N_CORES = 8
HPC = N_HEADS // N_CORES  # heads per core = 2


def build(B=B_FULL, S=S_FULL, E=EMB, hpc=HPC, DH=HEAD_DIM, CH=512):
    """Build the per-core Bass program (same program on all 8 cores)."""
    assert hpc == 2
    SB = B * S
    DHC = hpc * DH          # per-core head dims (256)
    NE = E // 128           # e-tiles (contraction tiles)
    NEH = NE // 2
    NCH = S // CH           # 512-wide chunks per sequence
    KPC = CH // 128         # k-tiles per chunk (4)
    NST = S // 128          # 128-row s-tiles per sequence
    NOC = E // CH           # output chunks
    scale = 1.0 / math.sqrt(DH)

    nc = bacc.Bacc("TRN2", target_bir_lowering=False, debug=False,
                   num_devices=N_CORES)

    xT = nc.dram_tensor("xT", [E, SB], BF16, kind="ExternalInput")
    wqT = nc.dram_tensor("wqT", [E, DHC], BF16, kind="ExternalInput")
    wkT = nc.dram_tensor("wkT", [E, DHC], BF16, kind="ExternalInput")
    wvT = nc.dram_tensor("wvT", [E, DHC], BF16, kind="ExternalInput")
    woT = nc.dram_tensor("woT", [DHC, E], BF16, kind="ExternalInput")
    masks = nc.dram_tensor("masks", [128, 128], BF16, kind="ExternalInput")
    ones = nc.dram_tensor("ones", [128, 1], BF16, kind="ExternalInput")
    y = nc.dram_tensor("y", [SB, E], BF16, kind="ExternalOutput")

    with tile.TileContext(nc) as tc:
        with (
            tc.tile_pool(name="wpool", bufs=1) as wpool,
            tc.tile_pool(name="xtp", bufs=2) as xtp,
            tc.tile_pool(name="qpool", bufs=1) as qpool,
            tc.tile_pool(name="kvpool", bufs=2) as kvpool,
            tc.tile_pool(name="ctxup", bufs=2) as ctxup,
            tc.tile_pool(name="expp", bufs=4) as expp,
            tc.tile_pool(name="denp", bufs=2) as denp,
            tc.tile_pool(name="dramp", bufs=3, space="DRAM") as dramp,
            tc.tile_pool(name="yp", bufs=4) as yp,
            tc.tile_pool(name="ps_sp", bufs=2, space="PSUM") as ps_sp,
            tc.tile_pool(name="ps_av", bufs=1, space="PSUM") as ps_av,
            tc.tile_pool(name="ps_dn", bufs=1, space="PSUM") as ps_dn,
            tc.tile_pool(name="ps_pj", bufs=2, space="PSUM") as ps_pj,
        ):
            # Resident weights / constants
            wq_sb = wpool.tile([128, NE, DHC], BF16, tag="wq")
            wk_sb = wpool.tile([128, NE, DHC], BF16, tag="wk")
            wv_sb = wpool.tile([128, NE, DHC], BF16, tag="wv")
            wo_sb = wpool.tile([128, hpc, E], BF16, tag="wo")
            xT_r = xT.rearrange("(t p) s -> p t s", p=128)
            wq_r = wqT.rearrange("(t p) d -> p t d", p=128)
            wk_r = wkT.rearrange("(t p) d -> p t d", p=128)
            # halves: the first Q/K accumulation consumes e-tiles in order
            nc.sync.dma_start(wq_sb[:, 0:NEH, :], wq_r[:, 0:NEH, :])
            nc.sync.dma_start(wk_sb[:, 0:NEH, :], wk_r[:, 0:NEH, :])
            nc.sync.dma_start(wq_sb[:, NEH:NE, :], wq_r[:, NEH:NE, :])
            nc.sync.dma_start(wk_sb[:, NEH:NE, :], wk_r[:, NEH:NE, :])
            # prefetch the first x chunk ahead of the remaining weights
            x0a = xtp.tile([128, NEH, CH], BF16, tag="xta", name="x0a")
            nc.sync.dma_start(x0a[:], xT_r[:, 0:NEH, 0:CH])
            x0b = xtp.tile([128, NEH, CH], BF16, tag="xtb", name="x0b")
            nc.sync.dma_start(x0b[:], xT_r[:, NEH:NE, 0:CH])
            xpre = ((0, 0), x0a, x0b)
            nc.sync.dma_start(wv_sb[:], wvT.rearrange("(t p) d -> p t d", p=128))
            nc.sync.dma_start(wo_sb[:], woT.rearrange("(h p) e -> p h e", p=128))
            mask_sb = wpool.tile([128, 128], BF16, tag="mask")
            nc.sync.dma_start(mask_sb[:], masks[:, :])
            ones_sb = wpool.tile([128, 1], BF16, tag="ones")
            nc.sync.dma_start(ones_sb[:], ones[:, :])

            evict_parity = [0]

            def emit_proj_tile(pctxn, st, oc, ps0):
                """One output tile [128 q, CH]: both heads accumulated into one
                PSUM bank, plain-copy evict (alternating ACT/DVE), y DMA."""
                p = ps_pj.tile([128, CH], F32, tag="pj")
                o0 = oc * CH
                nc.tensor.matmul(p[:], pctxn[:, 0, st * 128:(st + 1) * 128],
                                 wo_sb[:, 0, o0:o0 + CH], start=True, stop=False)
                nc.tensor.matmul(p[:], pctxn[:, 1, st * 128:(st + 1) * 128],
                                 wo_sb[:, 1, o0:o0 + CH], start=False, stop=True)
                ysb = yp.tile([128, CH], BF16, tag="ysb")
                if evict_parity[0] % 3 == 0:
                    nc.scalar.copy(ysb[:], p[:])
                else:
                    nc.vector.tensor_copy(ysb[:], p[:])
                evict_parity[0] += 1
                nc.gpsimd.dma_start(
                    y[ps0 + st * 128:ps0 + (st + 1) * 128, o0:o0 + CH], ysb[:])

            # pending proj work from the previous batch's last chunk:
            # list of (ctxn_tile, st, ps0) emitted as filler during phase A
            pending = []

            for b in range(B):
                s0 = b * S
                qT = qpool.tile([128, hpc, S], BF16, tag="qT")
                ctxTn = qpool.tile([128, hpc, S], BF16, tag="ctxn")
                kT = kvpool.tile([128, hpc, S], BF16, tag="kT")
                v_sb = kvpool.tile([128, NST, DHC], BF16, tag="v")

                # ---------------- Phase A: Q/K/V projections -------------
                fillers = list(pending)
                pending = []
                fi = 0
                n_groups = NCH * (2 * hpc + KPC)
                gi = 0

                def maybe_fill_a():
                    nonlocal fi, gi
                    gi += 1
                    while fi < len(fillers) and fi + 1 <= (
                            len(fillers) * gi + n_groups - 1) // n_groups:
                        pctxn, st, ps0, oc = fillers[fi]
                        emit_proj_tile(pctxn, st, oc, ps0)
                        fi += 1

                for ch in range(NCH):
                    c0 = ch * CH
                    if xpre is not None and xpre[0] == (b, ch):
                        xta, xtb = xpre[1], xpre[2]
                    else:
                        xta = xtp.tile([128, NEH, CH], BF16, tag="xta")
                        nc.sync.dma_start(xta[:],
                                          xT_r[:, 0:NEH, s0 + c0:s0 + c0 + CH])
                        xtb = xtp.tile([128, NEH, CH], BF16, tag="xtb")
                        nc.sync.dma_start(xtb[:],
                                          xT_r[:, NEH:NE, s0 + c0:s0 + c0 + CH])
                    if ch + 1 < NCH or b + 1 < B:
                        nb_, nch = (b, ch + 1) if ch + 1 < NCH else (b + 1, 0)
                        n0 = nb_ * S + nch * CH
                        xna = xtp.tile([128, NEH, CH], BF16, tag="xta",
                                       name="xna")
                        nc.sync.dma_start(xna[:], xT_r[:, 0:NEH, n0:n0 + CH])
                        xnb = xtp.tile([128, NEH, CH], BF16, tag="xtb",
                                       name="xnb")
                        nc.sync.dma_start(xnb[:], xT_r[:, NEH:NE, n0:n0 + CH])
                        xpre = ((nb_, nch), xna, xnb)
                    else:
                        xpre = None

                    def xslice(et, lo=None, hi=None):
                        t = xta if et < NEH else xtb
                        e = et if et < NEH else et - NEH
                        if lo is None:
                            return t[:, e, :]
                        return t[:, e, lo:hi]

                    for h in range(hpc):
                        qp = ps_pj.tile([128, CH], F32, tag="pj")
                        for et in range(NE):
                            nc.tensor.matmul(
                                qp[:], wq_sb[:, et, h * DH:(h + 1) * DH],
                                xslice(et),
                                start=(et == 0), stop=(et == NE - 1))
                        nc.scalar.activation(qT[:, h, c0:c0 + CH], qp[:],
                                             AF.Identity, scale=scale)
                        maybe_fill_a()
                        kp = ps_pj.tile([128, CH], F32, tag="pj")
                        for et in range(NE):
                            nc.tensor.matmul(
                                kp[:], wk_sb[:, et, h * DH:(h + 1) * DH],
                                xslice(et),
                                start=(et == 0), stop=(et == NE - 1))
                        nc.scalar.activation(kT[:, h, c0:c0 + CH], kp[:],
                                             AF.Identity)
                        maybe_fill_a()
                    for st in range(KPC):
                        vp = ps_pj.tile([128, DHC], F32, tag="pj")
                        for et in range(NE):
                            nc.tensor.matmul(
                                vp[:], xslice(et, st * 128, (st + 1) * 128),
                                wv_sb[:, et, :],
                                start=(et == 0), stop=(et == NE - 1))
                        nc.scalar.activation(v_sb[:, ch * KPC + st, :], vp[:],
                                             AF.Identity)
                        maybe_fill_a()
                # any leftover fillers
                while fi < len(fillers):
                    pctxn, st, ps0, oc = fillers[fi]
                    emit_proj_tile(pctxn, st, oc, ps0)
                    fi += 1

                # ------- Phase B: attention, heads interleaved ----------
                for g in range(NCH):
                    nk = KPC * (g + 1)
                    # proj fillers for chunk g-1 of this batch
                    gfill = []
                    if g > 0:
                        for st in range((g - 1) * KPC, g * KPC):
                            for oc in range(NOC):
                                gfill.append((st, oc))
                    gfi = 0
                    avp = [ps_av.tile([128, CH], F32, tag=f"av{h}",
                                    name=f"av{h}")
                           for h in range(hpc)]
                    dnp = [ps_dn.tile([1, CH], F32, tag=f"dn{h}",
                                      name=f"dn{h}")
                           for h in range(hpc)]
                    for kt in range(nk):
                        j = kt - (nk - KPC)
                        off = 128 * j if j > 0 else 0
                        sps = []
                        for h in range(hpc):
                            sp = ps_sp.tile([128, CH], F32, tag="sp",
                                            name=f"sp{h}")
                            nc.tensor.matmul(
                                sp[:, off:],
                                kT[:, h, kt * 128:(kt + 1) * 128],
                                qT[:, h, g * CH + off:(g + 1) * CH],
                                start=True, stop=True)
                            sps.append(sp)
                        # proj filler pairs for this step
                        while gfi < len(gfill) and gfi + 1 <= (
                                len(gfill) * (kt + 1) + nk - 1) // nk:
                            st, oc = gfill[gfi]
                            emit_proj_tile(ctxTn, st, oc, s0)
                            gfi += 1
                        for h in range(hpc):
                            sp = sps[h]
                            if j >= 0:
                                # mask col c: masked iff c < p (strict tri);
                                # only the first 128 cols of the suffix can hit
                                nc.vector.tensor_add(sp[:, off:off + 128],
                                                     sp[:, off:off + 128],
                                                     mask_sb[:, :])
                            ex = expp.tile([128, CH], BF16, tag="ex")
                            nc.scalar.activation(ex[:, off:], sp[:, off:],
                                                 AF.Exp)
                            nc.tensor.matmul(
                                avp[h][:, off:],
                                v_sb[:, kt, h * DH:(h + 1) * DH],
                                ex[:, off:],
                                start=(kt == 0), stop=(kt == nk - 1),
                                skip_group_check=True)
                            nc.tensor.matmul(
                                dnp[h][:, off:], ones_sb[:], ex[:, off:],
                                start=(kt == 0), stop=(kt == nk - 1),
                                skip_group_check=True)
                    # ---- per-head: evict ctx, build 1/den row, normalize ----
                    for h in range(hpc):
                        ctxu = ctxup.tile([128, CH], BF16, tag=f"ctxu{h}")
                        nc.scalar.copy(ctxu[:], avp[h][:])  # frees av bank
                        den_ch = denp.tile([1, CH], F32, tag=f"den_ch{h}")
                        nc.scalar.copy(den_ch[:], dnp[h][:])  # frees dn bank
                        # DRAM round-trip: row -> [128,4] -> recip -> row
                        den_d = dramp.tile([1, CH], F32, tag="den_d")
                        nc.gpsimd.dma_start(den_d[:], den_ch[:])
                        den_t = denp.tile([128, KPC], F32, tag="den_t")
                        nc.gpsimd.dma_start(
                            den_t[:],
                            den_d[:].rearrange("p (j q) -> (p q) j", j=KPC))
                        rden_t = denp.tile([128, KPC], F32, tag="rden_t")
                        nc.vector.reciprocal(rden_t[:], den_t[:])
                        rd_d = dramp.tile([1, CH], F32, tag="rd_d")
                        nc.gpsimd.dma_start(
                            rd_d[:].rearrange("p (j q) -> (p q) j", j=KPC),
                            rden_t[:])
                        rden_row = denp.tile([1, CH], F32, tag="rden_row")
                        nc.gpsimd.dma_start(rden_row[:], rd_d[:])
                        rdenb = denp.tile([128, CH], F32, tag="rdenb")
                        nc.gpsimd.partition_broadcast(rdenb[:], rden_row[:])
                        nc.vector.tensor_tensor(
                            ctxTn[:, h, g * CH:(g + 1) * CH], ctxu[:],
                            rdenb[:], op=ALU.mult)
                # last chunk's proj becomes filler for the next batch
                for st in range((NCH - 1) * KPC, NCH * KPC):
                    for oc in range(NOC):
                        pending.append((ctxTn, st, s0, oc))
            # tail: final batch's last-chunk proj
            for pctxn, st, ps0, oc in pending:
                emit_proj_tile(pctxn, st, oc, ps0)
    nc.finalize()
    return nc


def host_consts(CH=512):
    p = np.arange(128)[:, None]
    c = np.arange(128)[None, :]
    masks = np.where(c < p, np.float32(NEG), np.float32(0.0))
    return {
        "masks": np.ascontiguousarray(masks.astype(ml_dtypes.bfloat16)),
        "ones": np.ones((128, 1), dtype=ml_dtypes.bfloat16),
    }


def host_inputs(x, Wq, Wk, Wv, Wo, B=B_FULL, S=S_FULL, E=EMB, hpc=HPC,
                DH=HEAD_DIM, CH=512):
    """Shard + lay out the full inputs for the 8 cores (bf16)."""
    SB = B * S
    DHC = hpc * DH
    xT = np.ascontiguousarray(x.reshape(SB, E).T.astype(ml_dtypes.bfloat16))
    consts = host_consts(CH)

    in_maps = []
    for c in range(N_CORES):
        lo, hi = c * DHC, (c + 1) * DHC
        in_maps.append({
            "xT": xT,
            "wqT": np.ascontiguousarray(Wq[lo:hi, :].T.astype(ml_dtypes.bfloat16)),
            "wkT": np.ascontiguousarray(Wk[lo:hi, :].T.astype(ml_dtypes.bfloat16)),
            "wvT": np.ascontiguousarray(Wv[lo:hi, :].T.astype(ml_dtypes.bfloat16)),
            "woT": np.ascontiguousarray(Wo[:, lo:hi].T.astype(ml_dtypes.bfloat16)),
            **consts,
        })
    return in_maps


def kernel(x, Wq, Wk, Wv, Wo, bo):
    x = np.asarray(x, dtype=np.float32)
    Wq = np.asarray(Wq, dtype=np.float32)
    Wk = np.asarray(Wk, dtype=np.float32)
    Wv = np.asarray(Wv, dtype=np.float32)
    Wo = np.asarray(Wo, dtype=np.float32)
    bo = np.asarray(bo, dtype=np.float32)

    nc = build()
    in_maps = host_inputs(x, Wq, Wk, Wv, Wo)
    res = run_bass_kernel_spmd(nc, in_maps, list(range(N_CORES)))
    y = res.results[0]["y"].astype(np.float64)
    for c in range(1, N_CORES):
        y += res.results[c]["y"].astype(np.float64)
    y = (y + bo).astype(np.float32)
    return y.reshape(B_FULL, S_FULL, EMB)


# revision 5
# speedup vs baseline: 1.1046x; 1.0449x over previous
"""Trainium2 Bass kernel: cached causal self-attention (dense transformer block).

Full module: y = CausalAttn(x; Wq, Wk, Wv) @ Wo.T + bo with
  B=4, S=2048, E=2048, H=16 heads, Dh=128, fp32 inputs.

Distribution: 8-way tensor parallel over heads (2 heads per NeuronCore).
Each core computes Q/K/V projections for its 2 heads, causal-softmax
attention, and a partial output projection; the host sums the 8 partials
and adds the bias.

All matmul operands are bf16 (PSUM accumulation stays fp32): same PE
streaming rate as float32r but Fast-Weight-Load halves the LDWEIGHTS
cost, SBUF/DMA traffic halves, and power throttling drops.  End-to-end
rel err ~5e-3 (validated offline), well inside the 2e-2 gate.

Layout: x pre-transposed on host (xT [E, B*S]); scores computed
transposed (sT[k, q]) so exp(sT) feeds attn@V directly with no on-chip
transpose.  Softmax denominators come from a ones-vector matmul
accumulated in PSUM.

Schedule (the point of this version): the two heads' attention k-tile
loops are interleaved so the PE never waits on the scalar engine's exp
 -- per k-tile step the PE runs s0,s1,[proj filler],av0,dn0,av1,dn1
while ACT runs exp0,exp1 of the previous step's scores.  The output
projection accumulates BOTH heads into one PSUM bank (ctx is
pre-normalized by 1/den), turning the eviction into a plain copy, and
its matmul pairs are spread through the attention steps and the next
batch's QKV phase as PE filler work.  1/den is produced per (g,h) by a
DMA round-trip through DRAM (transpose to [128,4], DVE reciprocal,
transpose back, partition-broadcast) entirely on the idle gpsimd/DMA
path.
"""

import math

import ml_dtypes
import numpy as np

import concourse.bacc as bacc
import concourse.mybir as mybir
import concourse.tile as tile
from concourse.bass_utils import run_bass_kernel_spmd

F32 = mybir.dt.float32
BF16 = mybir.dt.bfloat16
AF = mybir.ActivationFunctionType
ALU = mybir.AluOpType

NEG = -1.0e30

# Full-problem constants
EMB = 2048
N_HEADS = 16
HEAD_DIM = 128
B_FULL = 4
S_FULL = 2048
N_CORES = 8
HPC = N_HEADS // N_CORES  # heads per core = 2


def build(B=B_FULL, S=S_FULL, E=EMB, hpc=HPC, DH=HEAD_DIM, CH=512):
    """Build the per-core Bass program (same program on all 8 cores)."""
    assert hpc == 2
    SB = B * S
    DHC = hpc * DH          # per-core head dims (256)
    NE = E // 128           # e-tiles (contraction tiles)
    NEH = NE // 2
    NCH = S // CH           # 512-wide chunks per sequence
    KPC = CH // 128         # k-tiles per chunk (4)
    NST = S // 128          # 128-row s-tiles per sequence
    NOC = E // CH           # output chunks
    scale = 1.0 / math.sqrt(DH)

    nc = bacc.Bacc("TRN2", target_bir_lowering=False, debug=False,
                   num_devices=N_CORES)

    xT = nc.dram_tensor("xT", [E, SB], BF16, kind="ExternalInput")
    wqT = nc.dram_tensor("wqT", [E, DHC], BF16, kind="ExternalInput")
    wkT = nc.dram_tensor("wkT", [E, DHC], BF16, kind="ExternalInput")
    wvT = nc.dram_tensor("wvT", [E, DHC], BF16, kind="ExternalInput")
    woT = nc.dram_tensor("woT", [DHC, E], BF16, kind="ExternalInput")
    masks = nc.dram_tensor("masks", [128, 128], BF16, kind="ExternalInput")
    ones = nc.dram_tensor("ones", [128, 1], BF16, kind="ExternalInput")
    y = nc.dram_tensor("y", [SB, E], BF16, kind="ExternalOutput")

    with tile.TileContext(nc) as tc:
        with (
            tc.tile_pool(name="wpool", bufs=1) as wpool,
            tc.tile_pool(name="xtp", bufs=2) as xtp,
            tc.tile_pool(name="qpool", bufs=1) as qpool,
            tc.tile_pool(name="kvpool", bufs=2) as kvpool,
            tc.tile_pool(name="ctxup", bufs=2) as ctxup,
            tc.tile_pool(name="expp", bufs=4) as expp,
            tc.tile_pool(name="denp", bufs=2) as denp,
            tc.tile_pool(name="dramp", bufs=3, space="DRAM") as dramp,
            tc.tile_pool(name="yp", bufs=4) as yp,
            tc.tile_pool(name="ps_sp", bufs=2, space="PSUM") as ps_sp,
            tc.tile_pool(name="ps_av", bufs=1, space="PSUM") as ps_av,
            tc.tile_pool(name="ps_dn", bufs=1, space="PSUM") as ps_dn,
            tc.tile_pool(name="ps_pj", bufs=2, space="PSUM") as ps_pj,
        ):
            # Resident weights / constants
            wq_sb = wpool.tile([128, NE, DHC], BF16, tag="wq")
            wk_sb = wpool.tile([128, NE, DHC], BF16, tag="wk")
            wv_sb = wpool.tile([128, NE, DHC], BF16, tag="wv")
            wo_sb = wpool.tile([128, hpc, E], BF16, tag="wo")
            xT_r = xT.rearrange("(t p) s -> p t s", p=128)
            wq_r = wqT.rearrange("(t p) d -> p t d", p=128)
            wk_r = wkT.rearrange("(t p) d -> p t d", p=128)
            # interleave weight halves with the first x chunk so the first
            # Q accumulation (wq + x) starts as early as possible
            x0a = xtp.tile([128, NEH, CH], BF16, tag="xta", name="x0a")
            x0b = xtp.tile([128, NEH, CH], BF16, tag="xtb", name="x0b")
            nc.sync.dma_start(wq_sb[:, 0:NEH, :], wq_r[:, 0:NEH, :])
            nc.sync.dma_start(x0a[:], xT_r[:, 0:NEH, 0:CH])
            nc.sync.dma_start(wq_sb[:, NEH:NE, :], wq_r[:, NEH:NE, :])
            nc.sync.dma_start(x0b[:], xT_r[:, NEH:NE, 0:CH])
            nc.sync.dma_start(wk_sb[:, 0:NEH, :], wk_r[:, 0:NEH, :])
            nc.sync.dma_start(wk_sb[:, NEH:NE, :], wk_r[:, NEH:NE, :])
            xpre = ((0, 0), x0a, x0b)
            nc.sync.dma_start(wv_sb[:], wvT.rearrange("(t p) d -> p t d", p=128))
            nc.sync.dma_start(wo_sb[:], woT.rearrange("(h p) e -> p h e", p=128))
            mask_sb = wpool.tile([128, 128], BF16, tag="mask")
            nc.sync.dma_start(mask_sb[:], masks[:, :])
            ones_sb = wpool.tile([128, 1], BF16, tag="ones")
            nc.sync.dma_start(ones_sb[:], ones[:, :])

            evict_parity = [0]

            def emit_proj_tile(pctxn, st, oc, ps0):
                """One output tile [128 q, CH]: both heads accumulated into one
                PSUM bank, plain-copy evict (alternating ACT/DVE), y DMA."""
                p = ps_pj.tile([128, CH], F32, tag="pj")
                o0 = oc * CH
                nc.tensor.matmul(p[:], pctxn[:, 0, st * 128:(st + 1) * 128],
                                 wo_sb[:, 0, o0:o0 + CH], start=True, stop=False)
                nc.tensor.matmul(p[:], pctxn[:, 1, st * 128:(st + 1) * 128],
                                 wo_sb[:, 1, o0:o0 + CH], start=False, stop=True)
                ysb = yp.tile([128, CH], BF16, tag="ysb")
                if evict_parity[0] % 3 == 0:
                    nc.scalar.copy(ysb[:], p[:])
                else:
                    nc.vector.tensor_copy(ysb[:], p[:])
                evict_parity[0] += 1
                nc.gpsimd.dma_start(
                    y[ps0 + st * 128:ps0 + (st + 1) * 128, o0:o0 + CH], ysb[:])

            # pending proj work from the previous batch's last chunk:
            # list of (ctxn_tile, st, ps0) emitted as filler during phase A
            pending = []

            for b in range(B):
                s0 = b * S
                qT = qpool.tile([128, hpc, S], BF16, tag="qT")
                ctxTn = qpool.tile([128, hpc, S], BF16, tag="ctxn")
                kT = kvpool.tile([128, hpc, S], BF16, tag="kT")
                v_sb = kvpool.tile([128, NST, DHC], BF16, tag="v")

                # ---------------- Phase A: Q/K/V projections -------------
                fillers = list(pending)
                pending = []
                fi = 0
                n_groups = NCH * (2 * hpc + KPC)
                gi = 0

                def maybe_fill_a():
                    nonlocal fi, gi
                    gi += 1
                    gd, nd = gi - 4, n_groups - 4
                    while fi < len(fillers) and gd >= 1 and fi + 1 <= (
                            len(fillers) * gd + nd - 1) // nd:
                        pctxn, st, ps0, oc = fillers[fi]
                        emit_proj_tile(pctxn, st, oc, ps0)
                        fi += 1

                for ch in range(NCH):
                    c0 = ch * CH
                    if xpre is not None and xpre[0] == (b, ch):
                        xta, xtb = xpre[1], xpre[2]
                    else:
                        xta = xtp.tile([128, NEH, CH], BF16, tag="xta")
                        nc.sync.dma_start(xta[:],
                                          xT_r[:, 0:NEH, s0 + c0:s0 + c0 + CH])
                        xtb = xtp.tile([128, NEH, CH], BF16, tag="xtb")
                        nc.sync.dma_start(xtb[:],
                                          xT_r[:, NEH:NE, s0 + c0:s0 + c0 + CH])
                    if ch + 1 < NCH or b + 1 < B:
                        nb_, nch = (b, ch + 1) if ch + 1 < NCH else (b + 1, 0)
                        n0 = nb_ * S + nch * CH
                        xna = xtp.tile([128, NEH, CH], BF16, tag="xta",
                                       name="xna")
                        nc.sync.dma_start(xna[:], xT_r[:, 0:NEH, n0:n0 + CH])
                        xnb = xtp.tile([128, NEH, CH], BF16, tag="xtb",
                                       name="xnb")
                        nc.sync.dma_start(xnb[:], xT_r[:, NEH:NE, n0:n0 + CH])
                        xpre = ((nb_, nch), xna, xnb)
                    else:
                        xpre = None

                    def xslice(et, lo=None, hi=None):
                        t = xta if et < NEH else xtb
                        e = et if et < NEH else et - NEH
                        if lo is None:
                            return t[:, e, :]
                        return t[:, e, lo:hi]

                    for h in range(hpc):
                        qp = ps_pj.tile([128, CH], F32, tag="pj")
                        for et in range(NE):
                            nc.tensor.matmul(
                                qp[:], wq_sb[:, et, h * DH:(h + 1) * DH],
                                xslice(et),
                                start=(et == 0), stop=(et == NE - 1))
                        nc.scalar.activation(qT[:, h, c0:c0 + CH], qp[:],
                                             AF.Identity, scale=scale)
                        maybe_fill_a()
                        kp = ps_pj.tile([128, CH], F32, tag="pj")
                        for et in range(NE):
                            nc.tensor.matmul(
                                kp[:], wk_sb[:, et, h * DH:(h + 1) * DH],
                                xslice(et),
                                start=(et == 0), stop=(et == NE - 1))
                        nc.scalar.activation(kT[:, h, c0:c0 + CH], kp[:],
                                             AF.Identity)
                        maybe_fill_a()
                    for st in range(KPC):
                        vp = ps_pj.tile([128, DHC], F32, tag="pj")
                        for et in range(NE):
                            nc.tensor.matmul(
                                vp[:], xslice(et, st * 128, (st + 1) * 128),
                                wv_sb[:, et, :],
                                start=(et == 0), stop=(et == NE - 1))
                        nc.scalar.activation(v_sb[:, ch * KPC + st, :], vp[:],
                                             AF.Identity)
                        maybe_fill_a()
                # any leftover fillers
                while fi < len(fillers):
                    pctxn, st, ps0, oc = fillers[fi]
                    emit_proj_tile(pctxn, st, oc, ps0)
                    fi += 1

                # ------- Phase B: attention, heads interleaved ----------
                for g in range(NCH):
                    nk = KPC * (g + 1)
                    # proj fillers for chunk g-1 of this batch
                    gfill = []
                    if g > 0:
                        for st in range((g - 1) * KPC, g * KPC):
                            for oc in range(NOC):
                                gfill.append((st, oc))
                    gfi = 0
                    avp = [ps_av.tile([128, CH], F32, tag=f"av{h}",
                                    name=f"av{h}")
                           for h in range(hpc)]
                    dnp = [ps_dn.tile([1, CH], F32, tag=f"dn{h}",
                                      name=f"dn{h}")
                           for h in range(hpc)]
                    for kt in range(nk):
                        j = kt - (nk - KPC)
                        off = 128 * j if j > 0 else 0
                        sps = []
                        for h in range(hpc):
                            sp = ps_sp.tile([128, CH], F32, tag="sp",
                                            name=f"sp{h}")
                            nc.tensor.matmul(
                                sp[:, off:],
                                kT[:, h, kt * 128:(kt + 1) * 128],
                                qT[:, h, g * CH + off:(g + 1) * CH],
                                start=True, stop=True)
                            sps.append(sp)
                        # proj filler pairs for this step (delayed a few
                        # steps so ctxTn(g-1)'s 1/den DMA chain can land)
                        kd, nd = kt - 2, nk - 2
                        while gfi < len(gfill) and kd >= 0 and gfi + 1 <= (
                                len(gfill) * (kd + 1) + nd - 1) // nd:
                            st, oc = gfill[gfi]
                            emit_proj_tile(ctxTn, st, oc, s0)
                            gfi += 1
                        for h in range(hpc):
                            sp = sps[h]
                            if j >= 0:
                                # mask col c: masked iff c < p (strict tri);
                                # only the first 128 cols of the suffix can hit
                                nc.vector.tensor_add(sp[:, off:off + 128],
                                                     sp[:, off:off + 128],
                                                     mask_sb[:, :])
                            ex = expp.tile([128, CH], BF16, tag="ex")
                            nc.scalar.activation(ex[:, off:], sp[:, off:],
                                                 AF.Exp)
                            nc.tensor.matmul(
                                avp[h][:, off:],
                                v_sb[:, kt, h * DH:(h + 1) * DH],
                                ex[:, off:],
                                start=(kt == 0), stop=(kt == nk - 1),
                                skip_group_check=True)
                            nc.tensor.matmul(
                                dnp[h][:, off:], ones_sb[:], ex[:, off:],
                                start=(kt == 0), stop=(kt == nk - 1),
                                skip_group_check=True)
                    # ---- per-head: evict ctx, build 1/den row, normalize ----
                    for h in range(hpc):
                        ctxu = ctxup.tile([128, CH], BF16, tag=f"ctxu{h}")
                        nc.scalar.copy(ctxu[:], avp[h][:])  # frees av bank
                        den_ch = denp.tile([1, CH], F32, tag=f"den_ch{h}")
                        nc.scalar.copy(den_ch[:], dnp[h][:])  # frees dn bank
                        # DRAM round-trip: row -> [128,4] -> recip -> row
                        den_d = dramp.tile([1, CH], F32, tag="den_d")
                        nc.sync.dma_start(den_d[:], den_ch[:])
                        den_t = denp.tile([128, KPC], F32, tag="den_t")
                        nc.sync.dma_start(
                            den_t[:],
                            den_d[:].rearrange("p (j q) -> (p q) j", j=KPC))
                        rden_t = denp.tile([128, KPC], F32, tag="rden_t")
                        nc.vector.reciprocal(rden_t[:], den_t[:])
                        rd_d = dramp.tile([1, CH], F32, tag="rd_d")
                        nc.sync.dma_start(
                            rd_d[:].rearrange("p (j q) -> (p q) j", j=KPC),
                            rden_t[:])
                        rden_row = denp.tile([1, CH], F32, tag="rden_row")
                        nc.sync.dma_start(rden_row[:], rd_d[:])
                        rdenb = denp.tile([128, CH], F32, tag="rdenb")
                        nc.gpsimd.partition_broadcast(rdenb[:], rden_row[:])
                        nc.vector.tensor_tensor(
                            ctxTn[:, h, g * CH:(g + 1) * CH], ctxu[:],
                            rdenb[:], op=ALU.mult)
                # last chunk's proj becomes filler for the next batch
                for st in range((NCH - 1) * KPC, NCH * KPC):
                    for oc in range(NOC):
                        pending.append((ctxTn, st, s0, oc))
            # tail: final batch's last-chunk proj
            for pctxn, st, ps0, oc in pending:
                emit_proj_tile(pctxn, st, oc, ps0)
    nc.finalize()
    return nc


def host_consts(CH=512):
    p = np.arange(128)[:, None]
    c = np.arange(128)[None, :]
    masks = np.where(c < p, np.float32(NEG), np.float32(0.0))
    return {
        "masks": np.ascontiguousarray(masks.astype(ml_dtypes.bfloat16)),
        "ones": np.ones((128, 1), dtype=ml_dtypes.bfloat16),
    }


def host_inputs(x, Wq, Wk, Wv, Wo, B=B_FULL, S=S_FULL, E=EMB, hpc=HPC,
                DH=HEAD_DIM, CH=512):
    """Shard + lay out the full inputs for the 8 cores (bf16)."""
    SB = B * S
    DHC = hpc * DH
    xT = np.ascontiguousarray(x.reshape(SB, E).T.astype(ml_dtypes.bfloat16))
    consts = host_consts(CH)

    in_maps = []
    for c in range(N_CORES):
        lo, hi = c * DHC, (c + 1) * DHC
        in_maps.append({
            "xT": xT,
            "wqT": np.ascontiguousarray(Wq[lo:hi, :].T.astype(ml_dtypes.bfloat16)),
            "wkT": np.ascontiguousarray(Wk[lo:hi, :].T.astype(ml_dtypes.bfloat16)),
            "wvT": np.ascontiguousarray(Wv[lo:hi, :].T.astype(ml_dtypes.bfloat16)),
            "woT": np.ascontiguousarray(Wo[:, lo:hi].T.astype(ml_dtypes.bfloat16)),
            **consts,
        })
    return in_maps


def kernel(x, Wq, Wk, Wv, Wo, bo):
    x = np.asarray(x, dtype=np.float32)
    Wq = np.asarray(Wq, dtype=np.float32)
    Wk = np.asarray(Wk, dtype=np.float32)
    Wv = np.asarray(Wv, dtype=np.float32)
    Wo = np.asarray(Wo, dtype=np.float32)
    bo = np.asarray(bo, dtype=np.float32)

    nc = build()
    in_maps = host_inputs(x, Wq, Wk, Wv, Wo)
    res = run_bass_kernel_spmd(nc, in_maps, list(range(N_CORES)))
    y = res.results[0]["y"].astype(np.float64)
    for c in range(1, N_CORES):
        y += res.results[c]["y"].astype(np.float64)
    y = (y + bo).astype(np.float32)
    return y.reshape(B_FULL, S_FULL, EMB)
